# revision 8
# baseline (speedup 1.0000x reference)
"""Trainium2 Bass kernel for nn_CrossAttention (B=4, N=2048, E=768, H=8).

Sharding: 8 cores = 4 batches x 2 head-groups (4 heads of 96 dims each).
Each core computes its batch's attention for its 4 heads plus the partial
output projection; the host sums the two head-group partials per batch and
adds bo.

v2 design notes (vs the original baseline):
  - every matmul stationary operand is exactly 128 columns wide (bf16) so
    the compiler-automatic Fast Weight Load path engages: weight tensors
    are zero-padded per head to 128 dims, and V carries its ones column
    (for the softmax rowsum) inside the padded 128-block via the bias add.
  - the scalar engine runs ONLY the exp activations; the O^T copy runs on
    DVE, the rowsum reciprocal is computed in place on partition 96 and
    broadcast with a 1-row f32r matmul contracting on partition 96, so
    there is no partition-shift DMA in the normalization chain.
  - single software-pipelined emission: K/Q/V projection chunks and the
    output-projection tiles are "filler units" drained two-per-kv-iteration
    into the attention loop so the PE never idles (pstate protection).
  - PSUM budget is exactly 8 banks: tag s 2x[128,1024]f32, tag po
    1x[128,1024], tag x 2x[128,512]; the tail out-projection ping-pongs
    over the then-dead s/po/x tags.
  - input DMA is split between the gpsimd and sync rings in arrival-order
    chunks so the first projection matmul can start a few us in.
"""

import os
import sys
import types
from collections import deque

import numpy as np

# ---------------------------------------------------------------------------
# NTFF profile hook (the agent image's antenv lacks axon_hooks; degrade OK)
# ---------------------------------------------------------------------------
def _install_ntff_hook():
    if "antenv.axon_hooks" in sys.modules:
        return
    try:
        hooks = types.ModuleType("antenv.axon_hooks")
        hooks._hook = None
        hooks.set_axon_ntff_profile_hook = lambda h: setattr(hooks, "_hook", h)
        hooks.get_axon_ntff_profile_hook = lambda: hooks._hook
        sys.modules["antenv.axon_hooks"] = hooks
        import antenv

        antenv.axon_hooks = hooks
        from trn_agent_boot.trn_boot import _ntff_profile_via_ctypes

        so = "/opt/axon/libaxon_pjrt.so"
        if os.path.exists(so):
            hooks.set_axon_ntff_profile_hook(_ntff_profile_via_ctypes(so))
    except Exception:
        pass


_install_ntff_hook()

import concourse.bacc as bacc
import concourse.tile as tile
import concourse.mybir as mybir
from concourse import bass_utils
from concourse.alu_op_type import AluOpType

F32 = mybir.dt.float32
F32R = mybir.dt.float32r
BF16 = mybir.dt.bfloat16

B = 4
NQ = 2048
NKV = 2048
E = 768
H_LOCAL = 4  # heads per core
HD = 96  # head dim
HP = 128  # padded head dim (FWL wants 128-wide stationaries)
D = H_LOCAL * HD  # 384 local proj dim
DP = H_LOCAL * HP  # 512 padded local proj dim
ET = E // 128  # 6 contraction tiles
KV_T = NKV // 128  # 16 kv tiles
QT_T = NQ // 128  # 16 q tiles
INV_SQRT_E = 1.0 / float(np.sqrt(np.float32(E)))


def build_nc():
    nc = bacc.Bacc("TRN2", target_bir_lowering=False, debug=False)

    xq_t = nc.dram_tensor("xq_t", [E, NQ], BF16, kind="ExternalInput")
    xkv_t = nc.dram_tensor("xkv_t", [E, NKV], BF16, kind="ExternalInput")
    wq_t = nc.dram_tensor("wq_t", [E, DP], BF16, kind="ExternalInput")
    wk_t = nc.dram_tensor("wk_t", [E, DP], BF16, kind="ExternalInput")
    wv_t = nc.dram_tensor("wv_t", [E, DP], BF16, kind="ExternalInput")
    wo_t = nc.dram_tensor("wo_t", [D, E], BF16, kind="ExternalInput")
    bq = nc.dram_tensor("bq", [D], F32, kind="ExternalInput")
    bk = nc.dram_tensor("bk", [D], F32, kind="ExternalInput")
    bve = nc.dram_tensor("bve", [DP], F32, kind="ExternalInput")
    ones96 = nc.dram_tensor("ones96", [HP], BF16, kind="ExternalInput")
    out = nc.dram_tensor("out", [NQ, E], F32, kind="ExternalOutput")

    with tile.TileContext(nc) as tc:
        with (
            nc.allow_low_precision(reason="bf16 matmuls and f32r broadcast"),
            tc.tile_pool(name="persist", bufs=1) as persist,
            tc.tile_pool(name="psum", bufs=1, space="PSUM") as pp,
            tc.tile_pool(name="sb", bufs=1) as sb,
        ):
            # ---------------- persistent SBUF tensors ----------------
            KT = persist.tile([HD, H_LOCAL, NKV], BF16)  # K^T per head
            QT = persist.tile([HD, H_LOCAL, NQ], BF16)  # Q^T per head
            # V: [kv-token, kv-tile, head-block(96 v dims + ones + 31 zeros)]
            V = persist.tile([128, KV_T, DP], BF16)
            attn = persist.tile([HD, H_LOCAL, NQ], BF16)  # normalized attn^T
            wo_sb = persist.tile([HD, H_LOCAL, E], BF16)
            wq_sb = persist.tile([128, ET, DP], BF16)
            wk_sb = persist.tile([128, ET, DP], BF16)
            wv_sb = persist.tile([128, ET, DP], BF16)
            bq_sb = persist.tile([HD, H_LOCAL], F32)
            bk_sb = persist.tile([HD, H_LOCAL], F32)
            bv_sb = persist.tile([128, DP], F32)
            ones_sb = persist.tile([HD + 1, HP], BF16)  # row 96 = [1]*96+[0]*32
            xkv_sb = persist.tile([128, ET, NKV], BF16)
            xq_sb = persist.tile([128, ET, NQ], BF16)

            # ---------------- input DMAs, in arrival order ----------------
            # sync(SP) ring: (wk-e, xkv-e wave0) interleaved, consts, wv,
            # xkv wave1 -- the K/V-side critical path.
            for e in range(ET):
                nc.sync.dma_start(
                    wk_sb[:, e, :], wk_t[e * 128 : (e + 1) * 128, :]
                )
                nc.sync.dma_start(
                    xkv_sb[:, e, 0:1024], xkv_t[e * 128 : (e + 1) * 128, 0:1024]
                )
            nc.sync.dma_start(bk_sb[:], bk[:].rearrange("(h p) -> p h", p=HD))
            nc.sync.dma_start(bv_sb[:], bve[:].partition_broadcast(128))
            nc.sync.dma_start(
                ones_sb[HD : HD + 1, :], ones96[:].rearrange("(o n) -> o n", o=1)
            )
            for e in range(ET):
                nc.sync.dma_start(
                    wv_sb[:, e, :], wv_t[e * 128 : (e + 1) * 128, :]
                )
            for e in range(ET):
                nc.sync.dma_start(
                    xkv_sb[:, e, 1024:2048],
                    xkv_t[e * 128 : (e + 1) * 128, 1024:2048],
                )
            # scalar(Act) ring: (wq-e, xq-e wave0) interleaved, bq, wo,
            # xq wave1 -- dispatch cost lands before any exp work.
            for e in range(ET):
                nc.scalar.dma_start(
                    wq_sb[:, e, :], wq_t[e * 128 : (e + 1) * 128, :]
                )
                nc.scalar.dma_start(
                    xq_sb[:, e, 0:1024], xq_t[e * 128 : (e + 1) * 128, 0:1024]
                )
            nc.scalar.dma_start(bq_sb[:], bq[:].rearrange("(h p) -> p h", p=HD))
            nc.scalar.dma_start(
                wo_sb[:], wo_t[:].rearrange("(h p) n -> p h n", p=HD)
            )
            for e in range(ET):
                nc.scalar.dma_start(
                    xq_sb[:, e, 1024:2048], xq_t[e * 128 : (e + 1) * 128, 1024:2048]
                )

            # ---------------- psum slot helpers ----------------
            # tags: s 2x[128,1024], po 1x[128,1024], x 2x[128,512]: 8 banks
            def psum_s():
                return pp.tile([128, 1024], F32, tag="s", bufs=2, name="ps_s")

            def psum_po():
                return pp.tile([128, 1024], F32, tag="po", bufs=1, name="ps_po")

            def psum_x():
                return pp.tile([128, 512], F32, tag="x", bufs=2, name="ps_x")

            # ---------------- projection / outproj unit emitters ----------
            def kq_unit(w_sb, b_sb, dst, x_sb, h, c):
                # dst[:, h, c*512:(c+1)*512] = w_h @ x^T + b   (96 valid rows)
                ps = psum_x()
                for e in range(ET):
                    nc.tensor.matmul(
                        ps[:],
                        w_sb[:, e, h * HP : (h + 1) * HP],
                        x_sb[:, e, c * 512 : (c + 1) * 512],
                        start=(e == 0),
                        stop=(e == ET - 1),
                    )
                nc.vector.tensor_scalar_add(
                    out=dst[:, h, c * 512 : (c + 1) * 512],
                    in0=ps[0:HD, :],
                    scalar1=b_sb[:, h : h + 1],
                )

            def v_unit(t):
                # V[:, t, :] = x_t @ Wv^T + bve  (bve carries the ones column)
                ps = psum_x()
                for e in range(ET):
                    nc.tensor.matmul(
                        ps[:],
                        xkv_sb[:, e, t * 128 : (t + 1) * 128],
                        wv_sb[:, e, :],
                        start=(e == 0),
                        stop=(e == ET - 1),
                    )
                nc.vector.tensor_tensor(
                    out=V[:, t, :],
                    in0=ps[:],
                    in1=bv_sb[:],
                    op=AluOpType.add,
                )

            ob_i = [0]

            def op_unit(qt, fa, fb):
                # out[qt] = attn^T_qt.T @ Wo^T  (accumulate 4 heads)
                for h in range(H_LOCAL):
                    nc.tensor.matmul(
                        fa,
                        attn[:, h, qt * 128 : (qt + 1) * 128],
                        wo_sb[:, h, 0:512],
                        start=(h == 0),
                        stop=(h == H_LOCAL - 1),
                    )
                for h in range(H_LOCAL):
                    nc.tensor.matmul(
                        fb,
                        attn[:, h, qt * 128 : (qt + 1) * 128],
                        wo_sb[:, h, 512:768],
                        start=(h == 0),
                        stop=(h == H_LOCAL - 1),
                    )
                ob = sb.tile([128, E], F32, tag="ob", bufs=2, name="ob")
                nc.vector.tensor_copy(ob[:, 0:512], fa)
                nc.vector.tensor_copy(ob[:, 512:768], fb)
                ring = nc.sync if (ob_i[0] % 2 == 0) else nc.gpsimd
                ob_i[0] += 1
                ring.dma_start(out[qt * 128 : (qt + 1) * 128, :], ob[:])

            def op_unit_x(qt):
                fa = psum_x()
                fb = psum_x()
                op_unit(qt, fa[:], fb[:, 0:256])

            # ---------------- filler machinery ----------------
            fillerA = deque()  # projection units: (deadline_block, fn)
            fillerB = deque()  # outproj units for qc0 (unlocked at block 5)
            b_unlocked = [False]

            def drain_one():
                if fillerA:
                    fillerA.popleft()[1]()
                elif b_unlocked[0] and fillerB:
                    fillerB.popleft()()

            def drain_deadline(blk):
                while fillerA and fillerA[0][0] <= blk:
                    fillerA.popleft()[1]()

            # ---------------- attention block ----------------
            def norm_rest_for(o_sb, rs, h, qc):
                def norm_rest():
                    for n in range(2):
                        bcT = psum_x()
                        nc.tensor.matmul(
                            bcT[:],
                            ones_sb[HD : HD + 1, :],
                            rs[HD : HD + 1, n * 512 : (n + 1) * 512],
                            start=True,
                            stop=True,
                            tile_position=(96, 0),
                        )
                        nc.vector.tensor_tensor(
                            out=attn[
                                :,
                                h,
                                qc * 1024 + n * 512 : qc * 1024 + (n + 1) * 512,
                            ],
                            in0=o_sb[0:HD, n * 512 : (n + 1) * 512],
                            in1=bcT[0:HD, :],
                            op=AluOpType.mult,
                        )

                return norm_rest

            def attn_block(qc, h, prev_norm, blk):
                drain_deadline(blk)
                po = psum_po()
                p_prev = None
                for kv in range(KV_T):
                    s = psum_s()
                    for n in range(2):
                        nc.tensor.matmul(
                            s[:, n * 512 : (n + 1) * 512],
                            KT[:, h, kv * 128 : (kv + 1) * 128],
                            QT[
                                :,
                                h,
                                qc * 1024 + n * 512 : qc * 1024 + (n + 1) * 512,
                            ],
                            start=True,
                            stop=True,
                        )
                    p = sb.tile([128, 1024], BF16, tag="p", bufs=3, name="p")
                    nc.scalar.activation(
                        p[:], s[:], mybir.ActivationFunctionType.Exp,
                        scale=INV_SQRT_E,
                    )
                    if kv == 5 and prev_norm is not None:
                        prev_norm()
                    drain_one()
                    drain_one()
                    if p_prev is not None:
                        for n in range(2):
                            nc.tensor.matmul(
                                po[:, n * 512 : (n + 1) * 512],
                                V[:, kv - 1, h * HP : (h + 1) * HP],
                                p_prev[:, n * 512 : (n + 1) * 512],
                                start=(kv == 1),
                                stop=False,
                            )
                    p_prev = p
                for n in range(2):
                    nc.tensor.matmul(
                        po[:, n * 512 : (n + 1) * 512],
                        V[:, KV_T - 1, h * HP : (h + 1) * HP],
                        p_prev[:, n * 512 : (n + 1) * 512],
                        start=False,
                        stop=True,
                    )
                o_sb = sb.tile([HD + 1, 1024], F32, tag="osb", bufs=2, name="o_sb")
                nc.vector.tensor_copy(o_sb[:], po[0 : HD + 1, :])
                rs = sb.tile([HD + 1, 1024], BF16, tag="rs", bufs=2, name="rs")
                nc.vector.reciprocal(
                    rs[HD : HD + 1, :], o_sb[HD : HD + 1, :]
                )
                return norm_rest_for(o_sb, rs, h, qc)

            # ---------------- preamble ----------------
            kq_unit(wk_sb, bk_sb, KT, xkv_sb, 0, 0)
            kq_unit(wk_sb, bk_sb, KT, xkv_sb, 0, 1)
            kq_unit(wq_sb, bq_sb, QT, xq_sb, 0, 0)
            kq_unit(wq_sb, bq_sb, QT, xq_sb, 0, 1)

            # ---------------- filler queues ----------------
            # order matters: the first 16 fillers MUST be V(t0..15) so that
            # the two-per-kv drain in block 0 emits V(t) before PV uses it.
            for t in range(KV_T):
                fillerA.append((1, lambda t=t: v_unit(t)))
            fillerA.append((0, lambda: kq_unit(wk_sb, bk_sb, KT, xkv_sb, 0, 2)))
            fillerA.append((0, lambda: kq_unit(wk_sb, bk_sb, KT, xkv_sb, 0, 3)))
            for h in range(1, H_LOCAL):
                for c in range(4):
                    fillerA.append(
                        (h, lambda h=h, c=c: kq_unit(
                            wk_sb, bk_sb, KT, xkv_sb, h, c))
                    )
                for c in range(2):
                    fillerA.append(
                        (h, lambda h=h, c=c: kq_unit(
                            wq_sb, bq_sb, QT, xq_sb, h, c))
                    )
            for h in range(H_LOCAL):
                for c in range(2, 4):
                    fillerA.append(
                        (4 + h, lambda h=h, c=c: kq_unit(
                            wq_sb, bq_sb, QT, xq_sb, h, c))
                    )
            for qt in range(8):  # outproj for qc0
                fillerB.append(lambda qt=qt: op_unit_x(qt))

            # K(h0,c2): needed from S(h0,kv8); drained by kv1 (position 17).
            # Reorder so the two K(h0) chunks come right after V(t0..3):
            items = list(fillerA)
            vhead, k0c23, rest = items[:4], items[16:18], items[4:16] + items[18:]
            fillerA = deque(vhead + k0c23 + rest)

            # ---------------- main loop ----------------
            prev_norm = None
            for qc in range(2):
                for h in range(H_LOCAL):
                    blk = qc * 4 + h
                    if blk == 5:
                        b_unlocked[0] = True
                    prev_norm = attn_block(qc, h, prev_norm, blk)
            # tail: last norm, remaining fillers, outproj qc1
            prev_norm()
            while fillerA:
                fillerA.popleft()[1]()
            b_unlocked[0] = True
            while fillerB:
                fillerB.popleft()()
            for i, qt in enumerate(range(8, QT_T)):
                m = i % 3
                if m == 0:
                    op_unit_x(qt)
                elif m == 1:
                    t = psum_s()
                    op_unit(qt, t[:, 0:512], t[:, 512:768])
                else:
                    t = psum_po()
                    op_unit(qt, t[:, 0:512], t[:, 512:768])

    nc.compile()
    return nc


_NC_CACHE = None


def _prep_inputs(x_query, x_kv, Wq, bq, Wk, bk, Wv, bv, Wo, bo):
    import ml_dtypes

    bf16 = ml_dtypes.bfloat16

    def pad_w(W_sl):
        # [384, 768] -> [768, 4, 128] zero-padded, flattened to [768, 512]
        wp = np.zeros((E, H_LOCAL, HP), dtype=np.float32)
        wt = W_sl.T  # [768, 384]
        for h in range(H_LOCAL):
            wp[:, h, 0:HD] = wt[:, h * HD : (h + 1) * HD]
        return np.ascontiguousarray(wp.reshape(E, DP)).astype(bf16)

    in_maps = []
    for c in range(8):
        b, g = divmod(c, 2)
        sl = slice(g * D, (g + 1) * D)
        bve = np.zeros((DP,), dtype=np.float32)
        for h in range(H_LOCAL):
            bve[h * HP : h * HP + HD] = bv[sl][h * HD : (h + 1) * HD]
            bve[h * HP + HD] = 1.0
        o96 = np.zeros((HP,), dtype=bf16)
        o96[0:HD] = 1.0
        in_maps.append(
            {
                "xq_t": np.ascontiguousarray(x_query[b].T).astype(bf16),
                "xkv_t": np.ascontiguousarray(x_kv[b].T).astype(bf16),
                "wq_t": pad_w(Wq[sl, :]),
                "wk_t": pad_w(Wk[sl, :]),
                "wv_t": pad_w(Wv[sl, :]),
                "wo_t": np.ascontiguousarray(Wo[:, sl].T).astype(bf16),
                "bq": np.ascontiguousarray(bq[sl]),
                "bk": np.ascontiguousarray(bk[sl]),
                "bve": bve,
                "ones96": o96,
            }
        )
    return in_maps


def kernel(x_query, x_kv, Wq, bq, Wk, bk, Wv, bv, Wo, bo):
    global _NC_CACHE
    x_query = np.asarray(x_query, dtype=np.float32)
    x_kv = np.asarray(x_kv, dtype=np.float32)
    Wq = np.asarray(Wq, dtype=np.float32)
    Wk = np.asarray(Wk, dtype=np.float32)
    Wv = np.asarray(Wv, dtype=np.float32)
    Wo = np.asarray(Wo, dtype=np.float32)
    bq = np.asarray(bq, dtype=np.float32)
    bk = np.asarray(bk, dtype=np.float32)
    bv = np.asarray(bv, dtype=np.float32)
    bo = np.asarray(bo, dtype=np.float32)

    if _NC_CACHE is None:
        _NC_CACHE = build_nc()
    nc = _NC_CACHE

    in_maps = _prep_inputs(x_query, x_kv, Wq, bq, Wk, bk, Wv, bv, Wo, bo)

    trace = bool(int(os.environ.get("KERNEL_TRACE", "0")))
    res = bass_utils.run_bass_kernel_spmd(
        nc, in_maps, core_ids=list(range(8)), trace=trace
    )
    if trace:
        kernel.last_exec_time_ns = res.exec_time_ns
        kernel.last_results = res

    out = np.empty((B, NQ, E), dtype=np.float32)
    for b in range(B):
        out[b] = res.results[2 * b]["out"] + res.results[2 * b + 1]["out"] + bo
    return out


# revision 10
# speedup vs baseline: 1.2106x; 1.2106x over previous
"""Trainium2 Bass kernel for nn_CrossAttention (B=4, N=2048, E=768, H=8).

Sharding: 8 cores = 4 batches x 2 head-groups (4 heads of 96 dims each).
Each core computes its batch's attention for its 4 heads plus the partial
output projection; the host sums the two head-group partials per batch and
adds bo.

v2 design notes (vs the original baseline):
  - every matmul stationary operand is exactly 128 columns wide (bf16) so
    the compiler-automatic Fast Weight Load path engages: weight tensors
    are zero-padded per head to 128 dims, and V carries its ones column
    (for the softmax rowsum) inside the padded 128-block via the bias add.
  - the scalar engine runs ONLY the exp activations; the O^T copy runs on
    DVE, the rowsum reciprocal is computed in place on partition 96 and
    broadcast with a 1-row f32r matmul contracting on partition 96, so
    there is no partition-shift DMA in the normalization chain.
  - single software-pipelined emission: K/Q/V projection chunks and the
    output-projection tiles are "filler units" drained two-per-kv-iteration
    into the attention loop so the PE never idles (pstate protection).
  - PSUM budget is exactly 8 banks: tag s 2x[128,1024]f32, tag po
    1x[128,1024], tag x 2x[128,512]; the tail out-projection ping-pongs
    over the then-dead s/po/x tags.
  - input DMA is split between the gpsimd and sync rings in arrival-order
    chunks so the first projection matmul can start a few us in.
"""

import os
import sys
import types
from collections import deque

import numpy as np

# ---------------------------------------------------------------------------
# NTFF profile hook (the agent image's antenv lacks axon_hooks; degrade OK)
# ---------------------------------------------------------------------------
def _install_ntff_hook():
    if "antenv.axon_hooks" in sys.modules:
        return
    try:
        hooks = types.ModuleType("antenv.axon_hooks")
        hooks._hook = None
        hooks.set_axon_ntff_profile_hook = lambda h: setattr(hooks, "_hook", h)
        hooks.get_axon_ntff_profile_hook = lambda: hooks._hook
        sys.modules["antenv.axon_hooks"] = hooks
        import antenv

        antenv.axon_hooks = hooks
        from trn_agent_boot.trn_boot import _ntff_profile_via_ctypes

        so = "/opt/axon/libaxon_pjrt.so"
        if os.path.exists(so):
            hooks.set_axon_ntff_profile_hook(_ntff_profile_via_ctypes(so))
    except Exception:
        pass


_install_ntff_hook()

import concourse.bacc as bacc
import concourse.tile as tile
import concourse.mybir as mybir
from concourse import bass_utils
from concourse.alu_op_type import AluOpType

F32 = mybir.dt.float32
F32R = mybir.dt.float32r
BF16 = mybir.dt.bfloat16

B = 4
NQ = 2048
NKV = 2048
E = 768
H_LOCAL = 4  # heads per core
HD = 96  # head dim
HP = 128  # padded head dim (FWL wants 128-wide stationaries)
D = H_LOCAL * HD  # 384 local proj dim
DP = H_LOCAL * HP  # 512 padded local proj dim
ET = E // 128  # 6 contraction tiles
KV_T = NKV // 128  # 16 kv tiles
QT_T = NQ // 128  # 16 q tiles
INV_SQRT_E = 1.0 / float(np.sqrt(np.float32(E)))


def build_nc():
    nc = bacc.Bacc("TRN2", target_bir_lowering=False, debug=False)

    xq_t = nc.dram_tensor("xq_t", [E, NQ], BF16, kind="ExternalInput")
    xkv_t = nc.dram_tensor("xkv_t", [E, NKV], BF16, kind="ExternalInput")
    wq_t = nc.dram_tensor("wq_t", [E, DP], BF16, kind="ExternalInput")
    wk_t = nc.dram_tensor("wk_t", [E, DP], BF16, kind="ExternalInput")
    wv_t = nc.dram_tensor("wv_t", [E, DP], BF16, kind="ExternalInput")
    wo_t = nc.dram_tensor("wo_t", [D, E], BF16, kind="ExternalInput")
    bq = nc.dram_tensor("bq", [D], F32, kind="ExternalInput")
    bk = nc.dram_tensor("bk", [D], F32, kind="ExternalInput")
    bve = nc.dram_tensor("bve", [DP], F32, kind="ExternalInput")
    ones96 = nc.dram_tensor("ones96", [HP], BF16, kind="ExternalInput")
    out = nc.dram_tensor("out", [NQ, E], F32, kind="ExternalOutput")

    with tile.TileContext(nc) as tc:
        with (
            nc.allow_low_precision(reason="bf16 matmuls and f32r broadcast"),
            tc.tile_pool(name="persist", bufs=1) as persist,
            tc.tile_pool(name="psum", bufs=1, space="PSUM") as pp,
            tc.tile_pool(name="sb", bufs=1) as sb,
        ):
            # ---------------- persistent SBUF tensors ----------------
            KT = persist.tile([HD, H_LOCAL, NKV], BF16)  # K^T per head
            QT = persist.tile([HD, H_LOCAL, NQ], BF16)  # Q^T per head
            # V: [kv-token, kv-tile, head-block(96 v dims + ones + 31 zeros)]
            V = persist.tile([128, KV_T, DP], BF16)
            attn = persist.tile([HD, H_LOCAL, NQ], BF16)  # normalized attn^T
            wo_sb = persist.tile([HD, H_LOCAL, E], BF16)
            wq_sb = persist.tile([128, ET, DP], BF16)
            wk_sb = persist.tile([128, ET, DP], BF16)
            wv_sb = persist.tile([128, ET, DP], BF16)
            bq_sb = persist.tile([HD, H_LOCAL], F32)
            bk_sb = persist.tile([HD, H_LOCAL], F32)
            bv_sb = persist.tile([128, DP], F32)
            ones_sb = persist.tile([HD + 1, HP], BF16)  # row 96 = [1]*96+[0]*32
            xkv_sb = persist.tile([128, ET, NKV], BF16)
            xq_sb = persist.tile([128, ET, NQ], BF16)

            # ---------------- input DMAs, in arrival order ----------------
            # sync(SP) ring: (wk-e, xkv-e wave0) interleaved, consts, wv,
            # xkv wave1 -- the K/V-side critical path.
            for e in range(ET):
                nc.sync.dma_start(
                    wk_sb[:, e, :], wk_t[e * 128 : (e + 1) * 128, :]
                )
                nc.sync.dma_start(
                    xkv_sb[:, e, 0:512], xkv_t[e * 128 : (e + 1) * 128, 0:512]
                )
            for e in range(ET):
                nc.sync.dma_start(
                    xkv_sb[:, e, 512:1024], xkv_t[e * 128 : (e + 1) * 128, 512:1024]
                )
            nc.sync.dma_start(bk_sb[:], bk[:].rearrange("(h p) -> p h", p=HD))
            nc.sync.dma_start(bv_sb[:], bve[:].partition_broadcast(128))
            nc.sync.dma_start(
                ones_sb[HD : HD + 1, :], ones96[:].rearrange("(o n) -> o n", o=1)
            )
            for e in range(ET):
                nc.sync.dma_start(
                    wv_sb[:, e, :], wv_t[e * 128 : (e + 1) * 128, :]
                )
            for e in range(ET):
                nc.sync.dma_start(
                    xkv_sb[:, e, 1024:2048],
                    xkv_t[e * 128 : (e + 1) * 128, 1024:2048],
                )
            # scalar(Act) ring: (wq-e, xq-e wave0) interleaved, bq, wo,
            # xq wave1 -- dispatch cost lands before any exp work.
            for e in range(ET):
                nc.scalar.dma_start(
                    wq_sb[:, e, :], wq_t[e * 128 : (e + 1) * 128, :]
                )
                nc.scalar.dma_start(
                    xq_sb[:, e, 0:512], xq_t[e * 128 : (e + 1) * 128, 0:512]
                )
            for e in range(ET):
                nc.scalar.dma_start(
                    xq_sb[:, e, 512:1024], xq_t[e * 128 : (e + 1) * 128, 512:1024]
                )
            nc.scalar.dma_start(bq_sb[:], bq[:].rearrange("(h p) -> p h", p=HD))
            nc.scalar.dma_start(
                wo_sb[:], wo_t[:].rearrange("(h p) n -> p h n", p=HD)
            )
            for e in range(ET):
                nc.scalar.dma_start(
                    xq_sb[:, e, 1024:2048], xq_t[e * 128 : (e + 1) * 128, 1024:2048]
                )

            # ---------------- psum slot helpers ----------------
            # tags: s 2x[128,1024], po 1x[128,1024], x 2x[128,512]: 8 banks
            def psum_s():
                return pp.tile([128, 1024], F32, tag="s", bufs=2, name="ps_s")

            def psum_po():
                return pp.tile([128, 1024], F32, tag="po", bufs=1, name="ps_po")

            def psum_x():
                return pp.tile([128, 512], F32, tag="x", bufs=2, name="ps_x")

            # ---------------- projection / outproj unit emitters ----------
            def kq_unit(w_sb, b_sb, dst, x_sb, h, c):
                # dst[:, h, c*512:(c+1)*512] = w_h @ x^T + b   (96 valid rows)
                ps = psum_x()
                for e in range(ET):
                    nc.tensor.matmul(
                        ps[:],
                        w_sb[:, e, h * HP : (h + 1) * HP],
                        x_sb[:, e, c * 512 : (c + 1) * 512],
                        start=(e == 0),
                        stop=(e == ET - 1),
                    )
                nc.vector.tensor_scalar_add(
                    out=dst[:, h, c * 512 : (c + 1) * 512],
                    in0=ps[0:HD, :],
                    scalar1=b_sb[:, h : h + 1],
                )

            def v_unit(t):
                # V[:, t, :] = x_t @ Wv^T + bve  (bve carries the ones column)
                ps = psum_x()
                for e in range(ET):
                    nc.tensor.matmul(
                        ps[:],
                        xkv_sb[:, e, t * 128 : (t + 1) * 128],
                        wv_sb[:, e, :],
                        start=(e == 0),
                        stop=(e == ET - 1),
                    )
                nc.vector.tensor_tensor(
                    out=V[:, t, :],
                    in0=ps[:],
                    in1=bv_sb[:],
                    op=AluOpType.add,
                )

            ob_i = [0]

            def op_unit(qt, fa, fb):
                # out[qt] = attn^T_qt.T @ Wo^T  (accumulate 4 heads)
                for h in range(H_LOCAL):
                    nc.tensor.matmul(
                        fa,
                        attn[:, h, qt * 128 : (qt + 1) * 128],
                        wo_sb[:, h, 0:512],
                        start=(h == 0),
                        stop=(h == H_LOCAL - 1),
                    )
                for h in range(H_LOCAL):
                    nc.tensor.matmul(
                        fb,
                        attn[:, h, qt * 128 : (qt + 1) * 128],
                        wo_sb[:, h, 512:768],
                        start=(h == 0),
                        stop=(h == H_LOCAL - 1),
                    )
                ob = sb.tile([128, E], F32, tag="ob", bufs=2, name="ob")
                nc.vector.tensor_copy(ob[:, 0:512], fa)
                nc.vector.tensor_copy(ob[:, 512:768], fb)
                ob_i[0] += 1
                nc.sync.dma_start(out[qt * 128 : (qt + 1) * 128, :], ob[:])

            def op_unit_x(qt):
                fa = psum_x()
                fb = psum_x()
                op_unit(qt, fa[:], fb[:, 0:256])

            # ---------------- filler machinery ----------------
            fillerA = deque()  # projection units: (deadline_block, fn)
            fillerB = deque()  # outproj units for qc0 (unlocked at block 5)
            b_unlocked = [False]

            def drain_one():
                if fillerA:
                    fillerA.popleft()[1]()
                elif b_unlocked[0] and fillerB:
                    fillerB.popleft()()

            def drain_deadline(blk):
                while fillerA and fillerA[0][0] <= blk:
                    fillerA.popleft()[1]()

            # ---------------- attention block ----------------
            def norm_rest_for(o_sb, rs, h, qc):
                def norm_rest():
                    for n in range(2):
                        bcT = psum_x()
                        nc.tensor.matmul(
                            bcT[:],
                            ones_sb[HD : HD + 1, :],
                            rs[HD : HD + 1, n * 512 : (n + 1) * 512],
                            start=True,
                            stop=True,
                            tile_position=(96, 0),
                        )
                        nc.vector.tensor_tensor(
                            out=attn[
                                :,
                                h,
                                qc * 1024 + n * 512 : qc * 1024 + (n + 1) * 512,
                            ],
                            in0=o_sb[0:HD, n * 512 : (n + 1) * 512],
                            in1=bcT[0:HD, :],
                            op=AluOpType.mult,
                        )

                return norm_rest

            def attn_block(qc, h, prev_norm, blk):
                drain_deadline(blk)
                po = psum_po()
                p_prev = None
                for kv in range(KV_T):
                    s = psum_s()
                    for n in range(2):
                        nc.tensor.matmul(
                            s[:, n * 512 : (n + 1) * 512],
                            KT[:, h, kv * 128 : (kv + 1) * 128],
                            QT[
                                :,
                                h,
                                qc * 1024 + n * 512 : qc * 1024 + (n + 1) * 512,
                            ],
                            start=True,
                            stop=True,
                        )
                    p = sb.tile([128, 1024], BF16, tag="p", bufs=3, name="p")
                    nc.scalar.activation(
                        p[:], s[:], mybir.ActivationFunctionType.Exp,
                        scale=INV_SQRT_E,
                    )
                    if kv == 5 and prev_norm is not None:
                        prev_norm()
                    drain_one()
                    drain_one()
                    if p_prev is not None:
                        for n in range(2):
                            nc.tensor.matmul(
                                po[:, n * 512 : (n + 1) * 512],
                                V[:, kv - 1, h * HP : (h + 1) * HP],
                                p_prev[:, n * 512 : (n + 1) * 512],
                                start=(kv == 1),
                                stop=False,
                            )
                    p_prev = p
                for n in range(2):
                    nc.tensor.matmul(
                        po[:, n * 512 : (n + 1) * 512],
                        V[:, KV_T - 1, h * HP : (h + 1) * HP],
                        p_prev[:, n * 512 : (n + 1) * 512],
                        start=False,
                        stop=True,
                    )
                o_sb = sb.tile([HD + 1, 1024], F32, tag="osb", bufs=2, name="o_sb")
                nc.vector.tensor_copy(o_sb[:], po[0 : HD + 1, :])
                rs = sb.tile([HD + 1, 1024], BF16, tag="rs", bufs=2, name="rs")
                rf = sb.tile([HD + 1, 1024], F32, tag="rf", bufs=2, name="rf")
                rscr = sb.tile([HD + 1, 1024], F32, tag="rscr", bufs=2, name="rscr")
                nc.vector.reciprocal_approx_accurate(
                    out=rf[:], in_=o_sb[:], scratch=rscr[:]
                )
                nc.vector.tensor_copy(rs[HD : HD + 1, :], rf[HD : HD + 1, :])
                return norm_rest_for(o_sb, rs, h, qc)

            # ---------------- preamble ----------------
            kq_unit(wk_sb, bk_sb, KT, xkv_sb, 0, 0)
            kq_unit(wk_sb, bk_sb, KT, xkv_sb, 0, 1)
            kq_unit(wq_sb, bq_sb, QT, xq_sb, 0, 0)
            kq_unit(wq_sb, bq_sb, QT, xq_sb, 0, 1)

            # ---------------- filler queues ----------------
            # order matters: the first 16 fillers MUST be V(t0..15) so that
            # the two-per-kv drain in block 0 emits V(t) before PV uses it.
            for t in range(KV_T):
                fillerA.append((1, lambda t=t: v_unit(t)))
            fillerA.append((0, lambda: kq_unit(wk_sb, bk_sb, KT, xkv_sb, 0, 2)))
            fillerA.append((0, lambda: kq_unit(wk_sb, bk_sb, KT, xkv_sb, 0, 3)))
            for h in range(1, H_LOCAL):
                for c in range(4):
                    fillerA.append(
                        (h, lambda h=h, c=c: kq_unit(
                            wk_sb, bk_sb, KT, xkv_sb, h, c))
                    )
                for c in range(2):
                    fillerA.append(
                        (h, lambda h=h, c=c: kq_unit(
                            wq_sb, bq_sb, QT, xq_sb, h, c))
                    )
            for h in range(H_LOCAL):
                for c in range(2, 4):
                    fillerA.append(
                        (4 + h, lambda h=h, c=c: kq_unit(
                            wq_sb, bq_sb, QT, xq_sb, h, c))
                    )
            for qt in range(8):  # outproj for qc0
                fillerB.append(lambda qt=qt: op_unit_x(qt))

            # K(h0,c2): needed from S(h0,kv8); drained by kv1 (position 17).
            # Reorder so the two K(h0) chunks come right after V(t0..3):
            items = list(fillerA)
            vhead, k0c23, rest = items[:4], items[16:18], items[4:16] + items[18:]
            fillerA = deque(vhead + k0c23 + rest)

            # ---------------- main loop ----------------
            prev_norm = None
            for qc in range(2):
                for h in range(H_LOCAL):
                    blk = qc * 4 + h
                    if blk == 5:
                        b_unlocked[0] = True
                    prev_norm = attn_block(qc, h, prev_norm, blk)
            # tail: last norm, remaining fillers, outproj qc1
            prev_norm()
            while fillerA:
                fillerA.popleft()[1]()
            b_unlocked[0] = True
            while fillerB:
                fillerB.popleft()()
            for i, qt in enumerate(range(8, QT_T)):
                m = i % 3
                if m == 0:
                    op_unit_x(qt)
                elif m == 1:
                    t = psum_s()
                    op_unit(qt, t[:, 0:512], t[:, 512:768])
                else:
                    t = psum_po()
                    op_unit(qt, t[:, 0:512], t[:, 512:768])

    nc.compile()
    return nc


_NC_CACHE = None


def _prep_inputs(x_query, x_kv, Wq, bq, Wk, bk, Wv, bv, Wo, bo):
    import ml_dtypes

    bf16 = ml_dtypes.bfloat16

    def pad_w(W_sl):
        # [384, 768] -> [768, 4, 128] zero-padded, flattened to [768, 512]
        wp = np.zeros((E, H_LOCAL, HP), dtype=np.float32)
        wt = W_sl.T  # [768, 384]
        for h in range(H_LOCAL):
            wp[:, h, 0:HD] = wt[:, h * HD : (h + 1) * HD]
        return np.ascontiguousarray(wp.reshape(E, DP)).astype(bf16)

    in_maps = []
    for c in range(8):
        b, g = divmod(c, 2)
        sl = slice(g * D, (g + 1) * D)
        bve = np.zeros((DP,), dtype=np.float32)
        for h in range(H_LOCAL):
            bve[h * HP : h * HP + HD] = bv[sl][h * HD : (h + 1) * HD]
            bve[h * HP + HD] = 1.0
        o96 = np.zeros((HP,), dtype=bf16)
        o96[0:HD] = 1.0
        in_maps.append(
            {
                "xq_t": np.ascontiguousarray(x_query[b].T).astype(bf16),
                "xkv_t": np.ascontiguousarray(x_kv[b].T).astype(bf16),
                "wq_t": pad_w(Wq[sl, :]),
                "wk_t": pad_w(Wk[sl, :]),
                "wv_t": pad_w(Wv[sl, :]),
                "wo_t": np.ascontiguousarray(Wo[:, sl].T).astype(bf16),
                "bq": np.ascontiguousarray(bq[sl]),
                "bk": np.ascontiguousarray(bk[sl]),
                "bve": bve,
                "ones96": o96,
            }
        )
    return in_maps


def kernel(x_query, x_kv, Wq, bq, Wk, bk, Wv, bv, Wo, bo):
    global _NC_CACHE
    x_query = np.asarray(x_query, dtype=np.float32)
    x_kv = np.asarray(x_kv, dtype=np.float32)
    Wq = np.asarray(Wq, dtype=np.float32)
    Wk = np.asarray(Wk, dtype=np.float32)
    Wv = np.asarray(Wv, dtype=np.float32)
    Wo = np.asarray(Wo, dtype=np.float32)
    bq = np.asarray(bq, dtype=np.float32)
    bk = np.asarray(bk, dtype=np.float32)
    bv = np.asarray(bv, dtype=np.float32)
    bo = np.asarray(bo, dtype=np.float32)

    if _NC_CACHE is None:
        _NC_CACHE = build_nc()
    nc = _NC_CACHE

    in_maps = _prep_inputs(x_query, x_kv, Wq, bq, Wk, bk, Wv, bv, Wo, bo)

    trace = bool(int(os.environ.get("KERNEL_TRACE", "0")))
    res = bass_utils.run_bass_kernel_spmd(
        nc, in_maps, core_ids=list(range(8)), trace=trace
    )
    if trace:
        kernel.last_exec_time_ns = res.exec_time_ns
        kernel.last_results = res

    out = np.empty((B, NQ, E), dtype=np.float32)
    for b in range(B):
        out[b] = res.results[2 * b]["out"] + res.results[2 * b + 1]["out"] + bo
    return out
